# revision 9
# baseline (speedup 1.0000x reference)
"""2D-DCT (DCT-II, orthonormal) spatial transform on Trainium2, 8 NeuronCores.

Full input x [16,256,128,128] f32 -> out[b,c,k,v] = sum_hw Wy[k,h] Wx[v,w] x[b,c,h,w]
with Wy = Wx = 128-point orthonormal DCT-II matrix W.

Strategy (data-parallel, batch*channel sharded 4096 -> 512 images/core):
per image X: out = W @ X @ W.T, computed with two PE matmuls and zero
explicit transposes -- matmul(out, lhsT, rhs) = lhsT.T @ rhs transposes the
stationary operand for free. BOTH stages run in float32r (TF32-like 8e11m PE
mode, 1 cycle/row when the moving free dim is >=256) by streaming the same
duplicated weight tensor wr2 = [W.T | W.T] [128,256]:
  mm1: lhsT=X_i   (f32r), rhs=wr2 -> [Z^T|Z^T]  (PSUM, keep left half)
  mm2: lhsT=Z^T_i (f32r), rhs=wr2 -> [out|out]  (PSUM, keep left half)
256 PE cycles/img of matmul (vs 640 for the fp32-stage-1 variant) plus
2x128-col LDWEIGHTS/img that overlap streaming via the PE's dual weight
buffers. Rounding X, Z^T and W to tf32 gives ~4e-4 scale-relative error.
PSUM->SBUF copies are batched (4 images on ACT for stage 1, 2 on DVE for
stage 2) and PSUM banks are packed 8/8 so every engine sits under the HBM
roofline (~366 ns/img/core); the kernel is DMA-bound.
"""

import sys

for _p in ("/opt/trn_rl_repo", "/root/.axon_site/_ro/trn_rl_repo"):
    if _p not in sys.path:
        sys.path.insert(0, _p)

import numpy as np

N_CORES = 8
B, C, H, W = 16, 256, 128, 128
PER_CORE = B * C // N_CORES  # 512 images per core


def _dct_matrix(n: int) -> np.ndarray:
    v = np.arange(n, dtype=np.float64)[:, None]
    j = np.arange(n, dtype=np.float64)[None, :]
    f = np.cos(np.pi * (0.5 + j) * v / n) / np.sqrt(n)
    f *= np.where(v != 0, np.sqrt(2.0), 1.0)
    return f.astype(np.float32)


def _build_program(n_img: int, group: int = 8, xg_bufs: int = 4, og_bufs: int = 4,
                   p1_bufs: int = 2, p2_bufs: int = 4, zt_bufs: int = 3,
                   reps: int = 1):
    import contextlib

    import concourse.bacc as bacc_mod
    import concourse.mybir as mybir
    from concourse.tile import TileContext

    F32 = mybir.dt.float32
    F32R = mybir.dt.float32r

    nc = bacc_mod.Bacc()
    x = nc.declare_dram_parameter("x", [n_img, 128, 128], F32R, isOutput=False)
    wr2_p = nc.declare_dram_parameter("wr2", [128, 256], F32, isOutput=False)
    out = nc.declare_dram_parameter("out", [n_img, 128, 128], F32, isOutput=True)

    with TileContext(nc) as tc:
        with tc.tile_pool(name="consts", bufs=1) as cpool, \
             tc.tile_pool(name="xin", bufs=xg_bufs) as xpool, \
             tc.tile_pool(name="mid", bufs=zt_bufs) as zpool, \
             tc.tile_pool(name="oput", bufs=og_bufs) as opool, \
             tc.tile_pool(name="ps", bufs=1, space="PSUM") as pspool:
            wr2 = cpool.tile([128, 256], F32R)
            nc.gpsimd.dma_start(out=wr2, in_=wr2_p[:])

            # PE warm-up dummy: absorbs the wr2-DMA wait so no later
            # (self-loading) matmul needs more than one sync wait -- the
            # S3_LW struct can carry only one. Writes into the p2 rotation.
            pdum = pspool.tile([128, 2, 256], F32, tag="p2", bufs=p2_bufs)
            nc.tensor.matmul(pdum[:, 0, :], lhsT=wr2[:, :128], rhs=wr2,
                             start=True, stop=True)

            loop_ctx = tc.For_i(0, reps) if reps > 1 else contextlib.nullcontext()
            with loop_ctx:
                for g in range(n_img // group):
                    xg = xpool.tile([128, group, 128], F32R, tag="xg")
                    nc.sync.dma_start(
                        out=xg,
                        in_=x[g * group:(g + 1) * group].rearrange("i h w -> h i w"))
                    og = opool.tile([128, group, 128], F32, tag="og")
                    for q in range(group // 4):
                        p1 = pspool.tile([128, 4, 256], F32, tag="p1", bufs=p1_bufs)
                        for i in range(4):
                            nc.tensor.matmul(p1[:, i, :],
                                             lhsT=xg[:, q * 4 + i, :],
                                             rhs=wr2, start=True, stop=True)
                        zt = zpool.tile([128, 4, 128], F32R, tag="zt")
                        nc.scalar.copy(out=zt, in_=p1[:, :, :128])  # batched cast copy (ACT)
                        for h in range(2):
                            p2 = pspool.tile([128, 2, 256], F32, tag="p2", bufs=p2_bufs)
                            for i in range(2):
                                nc.tensor.matmul(p2[:, i, :], lhsT=zt[:, h * 2 + i, :],
                                                 rhs=wr2, start=True, stop=True)
                            nc.vector.tensor_copy(
                                out=og[:, q * 4 + h * 2: q * 4 + h * 2 + 2, :],
                                in_=p2[:, :, :128])
                    nc.sync.dma_start(
                        out=out[g * group:(g + 1) * group].rearrange("i h w -> h i w"),
                        in_=og)
    nc.finalize()
    return nc


_CACHE = {}


def kernel(x: np.ndarray) -> np.ndarray:
    from concourse.bass_utils import run_bass_kernel_spmd

    assert x.shape == (B, C, H, W), x.shape
    x = np.ascontiguousarray(x, dtype=np.float32)

    if "nc" not in _CACHE:
        _CACHE["nc"] = _build_program(PER_CORE)
    nc = _CACHE["nc"]

    wt = _dct_matrix(128).T.copy().astype(np.float32)       # WT[h,k] = W[k,h]
    wr2 = np.concatenate([wt, wt], axis=1).astype(np.float32)

    flat = x.reshape(B * C, H, W)
    in_maps = [
        {"x": flat[c * PER_CORE:(c + 1) * PER_CORE], "wr2": wr2}
        for c in range(N_CORES)
    ]
    res = run_bass_kernel_spmd(nc, in_maps, list(range(N_CORES)))
    out = np.concatenate([r["out"] for r in res.results], axis=0)
    return out.reshape(B, C, H, W).astype(np.float32)


if __name__ == "__main__":
    rng = np.random.default_rng(0)
    xs = rng.standard_normal((B, C, H, W), dtype=np.float32)
    o = kernel(xs)
    print("kernel output", o.shape, o.dtype)
